# revision 21
# baseline (speedup 1.0000x reference)
"""TRN2 Bass kernel for nn_SynthesisLayer (StyleGAN-style modulated 3D conv).

Math: for each sample b
  styles = w[b] @ affine_weight.T / sqrt(512) + affine_bias          [Cin]
  wmod   = weight * styles[None,:,None]                              [Co,Ci,27]
  dcoef  = rsqrt(sum_{ci,k} wmod^2 + 1e-8)                           [Co]
  y      = dcoef * conv3d(x[b], wmod, pad=1) + noise_const*ns + bias
  out    = clip(lrelu(y)*sqrt(2), -256, 256)

Device implementation (per core):
  - conv3d = shifted matmuls (contraction over Cin=128 on partitions)
    accumulated in PSUM, weights modulated on device by styles.
  - fp8 DoubleRow path: x is pre-split on the host into e4m3 hi+lo slabs;
    modulated (bf16-shipped) weights are split on device into e4m3 hi+lo.
    Each DoubleRow matmul computes two (weight, shifted-x) products per
    PSUM row at 0.5 cycles/row.  Per output element: 27 hi*hi products +
    27 w_hi*x_lo + 18 w_lo*x_hi corrections (w_lo of taps 18-26 dropped,
    rel err ~1.7% vs the 2e-2 budget) = 72 products = 36 matmuls per
    512-wide tile, vs 27 full-rate fp32r matmuls for the exact conv.
  - demod + noise fold into a per-partition scale (ACT Prelu) and a DVE
    scalar_tensor_tensor; demod squares/reductions run on the Pool engine
    off the DVE critical path.

Sharding: 8 cores = 4 samples x 2 D-halves. Each core gets zero-padded
fp8 hi/lo input slabs [128, NSLAB] (33-wide rows, 33-row slices, one-slice
D halo), computes output [128, 16*32*32], host reassembles. No collectives.
"""

import math
import os
import sys

for _p in ("/opt/trn_rl_repo", "/root/.axon_site/_ro/trn_rl_repo"):
    if os.path.isdir(_p) and _p not in sys.path:
        sys.path.insert(0, _p)

import numpy as np
import ml_dtypes

import concourse.mybir as mybir
from concourse import bacc
from concourse.ap import AP
from concourse.tile import TileContext
from concourse.bass_utils import run_bass_kernel_spmd

P = 128          # Cin = Cout = 128
TAPS = 27        # 3x3x3
NDROP = 9        # taps whose w_lo correction is dropped (18..26)
NLO = TAPS - NDROP
RES = 32
B = 4
W_DIM = 512
ROW = 33         # padded row width  (32 real + 1 zero)
SLICE = ROW * ROW  # 1089 padded slice (32 real rows + 1 zero row)
LEAD = 34        # leading zero guard (one row + one elem)
NSLICES = 18     # 16 output slices + 1 halo each side
BODY = NSLICES * SLICE
NSLAB = LEAD + BODY + 46   # tail guard; max AP end = 19637
DHALF = 16                 # output D slices per core
NOUT = DHALF * RES * RES   # 16384
DCH = 4                    # output D slices per pipelined input chunk
NSLABC = LEAD + (DCH + 2) * SLICE + 46  # 6614: chunk tile incl. halo+guards
NCHUNK = 512               # psum tile free size (one PSUM bank of fp32)
LRELU_ALPHA = 0.2
LRELU_GAIN = math.sqrt(2.0)
CLAMP = 256.0

f32 = mybir.dt.float32
f32r = mybir.dt.float32r
bf16 = mybir.dt.bfloat16
fp8 = mybir.dt.float8e4
DRMODE = mybir.MatmulPerfMode.DoubleRow
AF = mybir.ActivationFunctionType
E4 = ml_dtypes.float8_e4m3fn

# tap k = kd*9 + kh*3 + kw; shift of tap k relative to the tile center
TAP_OFF = [
    (kd - 1) * SLICE + (kh - 1) * ROW + (kw - 1)
    for kd in range(3) for kh in range(3) for kw in range(3)
]

_NC_CACHE = None
LAST_EXEC_NS = None


def _pair_ap(flat_ap, off, delta, inner_dims):
    """[[p],[delta,2],*inner_dims] AP at element offset `off` of a 2D AP."""
    dims = [list(flat_ap.ap[0]), [delta, 2]] + [list(d) for d in inner_dims]
    return AP(flat_ap.tensor, flat_ap.offset + off, dims)


def build_nc():
    nc = bacc.Bacc("TRN2", target_bir_lowering=False, debug=False, num_devices=8)
    pool = nc.engines[mybir.EngineType.Pool]

    xhi = nc.dram_tensor("xhi", [P, NSLAB], fp8, kind="ExternalInput")
    xlo = nc.dram_tensor("xlo", [P, NSLAB], fp8, kind="ExternalInput")
    wt = nc.dram_tensor("wt", [P, TAPS, P], bf16, kind="ExternalInput")
    # affwv = aff (4*128) | wv (5) | pad, one DMA for the whole style path
    affwv = nc.dram_tensor("affwv", [P, 520], bf16, kind="ExternalInput")
    # sm cols: 0=affine_bias 1=bias 2=noise_strength 3=lrelu_alpha 4=eps
    #          5=zero 6,7=ones
    sm = nc.dram_tensor("sm", [P, 8], f32, kind="ExternalInput")
    nz = nc.dram_tensor("nz", [1, NOUT], f32, kind="ExternalInput")
    y = nc.dram_tensor("y", [P, NOUT], f32, kind="ExternalOutput")

    with TileContext(nc) as tc:
        with (
            tc.tile_pool(name="big", bufs=1) as big,
            tc.tile_pool(name="small", bufs=1) as small,
            tc.tile_pool(name="nzp", bufs=6) as nzp,
            tc.tile_pool(name="xchunk", bufs=2) as xchunk,
            tc.tile_pool(name="outp", bufs=4) as outp,
            tc.tile_pool(name="cpsum", bufs=6, space="PSUM") as cpsum,
            tc.tile_pool(name="spsum", bufs=1, space="PSUM") as spsum,
        ):
            # dummy activation with no DMA deps: hoists the one-time
            # LoadActFuncSet (~1.3us) off the styles critical path
            dummy = small.tile([P, 1], f32)
            pool.memset(dummy[:], 0.0)
            nc.scalar.activation(dummy[:], dummy[:], AF.Identity, bias=dummy[:])
            nc.scalar.activation(dummy[:], dummy[:], AF.Sqrt, bias=dummy[:])
            nc.scalar.activation(
                dummy[:], dummy[:], AF.Prelu, bias=dummy[:], scale=1.0,
                alpha=dummy[:],
            )

            # ---- merged small loads first: the style path comes off these ----
            affwv_sb = small.tile([P, 520], bf16)
            nc.sync.dma_start(affwv_sb[:], affwv[:])
            sm_sb = small.tile([P, 8], f32)
            nc.sync.dma_start(sm_sb[:], sm[:])
            ab_sb = sm_sb[:, 0:1]
            bb_sb = sm_sb[:, 1:2]
            nsb_sb = sm_sb[:, 2:3]
            acol_sb = sm_sb[:, 3:4]
            epsc_sb = sm_sb[:, 4:5]
            zc_sb = sm_sb[:, 5:6]

            # weight DMA in tap chunks so quantization starts early
            wt_sb = big.tile([P, TAPS, P], bf16)
            WCH = [(0, 2), (2, 9), (9, 16), (16, 23), (23, TAPS)]
            CHUNKS = [(1, 1), (2, 3), (5, 4), (9, 4), (13, 4)]  # (a, n_out)
            xt1 = xchunk.tile([P, 2, NSLABC], fp8, tag="xchunk")
            wlen1 = LEAD + 3 * SLICE + 46
            step = 2 * SLICE
            nc.sync.dma_start(wt_sb[:, 0:2, :], wt[:, 0:2, :])
            nc.sync.dma_start(wt_sb[:, 2:9, :], wt[:, 2:9, :])
            nc.sync.dma_start(xt1[:, 1, 0:step], xhi[:, 0:step])
            nc.sync.dma_start(xt1[:, 1, step:wlen1], xhi[:, step:wlen1])
            for c0, c1 in WCH[2:]:
                nc.sync.dma_start(wt_sb[:, c0:c1, :], wt[:, c0:c1, :])
            nc.sync.dma_start(xt1[:, 0, 0:step], xlo[:, 0:step])
            nc.sync.dma_start(xt1[:, 0, step:wlen1], xlo[:, step:wlen1])

            # ---- styles = w[b] @ aff.T / sqrt(512) + affine_bias ----
            # N=1 matmuls are ISA-illegal; use N=2 and read col 0
            st_ps = spsum.tile([P, 2], f32, tag="st")
            for j in range(4):
                nc.tensor.matmul(
                    st_ps[:], affwv_sb[:, j * P : (j + 1) * P],
                    affwv_sb[:, 512 + j : 514 + j],
                    start=(j == 0), stop=(j == 3),
                )
            styles = small.tile([P, 1], f32)
            nc.scalar.activation(
                styles[:], st_ps[:, :1], AF.Identity,
                bias=ab_sb, scale=1.0 / math.sqrt(W_DIM),
            )

            # ---- modulated weights, split into fp8 hi + lo ----
            # wq slots 0..26 = hi taps, 27..48 = lo taps 0..21
            wq = big.tile([P, TAPS + NLO, P], fp8)
            for c0, c1 in WCH:
                nc.vector.tensor_scalar_mul(
                    wq[:, c0:c1, :], wt_sb[:, c0:c1, :], styles[:]
                )
            for c0, c1 in ((0, 6), (6, 12), (12, NLO)):
                nc.vector.scalar_tensor_tensor(
                    wq[:, TAPS + c0 : TAPS + c1, :], wt_sb[:, c0:c1, :],
                    styles[:], wq[:, c0:c1, :],
                    mybir.AluOpType.mult, mybir.AluOpType.subtract,
                )
            wq_flat = wq[:].rearrange("p a b -> p (a b)")

            # B_col = bias * sqrt(2)
            b_col = small.tile([P, 1], f32)
            nc.vector.tensor_scalar_mul(b_col[:], bb_sb, LRELU_GAIN)
            # noise gain = noise_strength * sqrt(2), per partition
            nsg = small.tile([P, 1], f32)
            nc.vector.tensor_scalar_mul(nsg[:], nsb_sb, LRELU_GAIN)
            s_col = small.tile([P, 1], f32)

            def demod_block():
                # ---- demod sums: v[co] = sum_ci styles^2 * (sum_k wt^2),
                # via 27 tiny PE matmuls sq[:,k,:]^T @ s2 accumulating in PSUM
                # (no DVE reduce on the critical path).  Issued after chunk 1
                # so the PE's in-order stream reaches these matmuls only once
                # their inputs are long since ready; the first epilogue waits
                # on s_col, covered by PSUM buffering
                sq_sb = big.tile([P, TAPS, P], bf16)  # [ci, k, co]
                for c0, c1 in WCH:
                    pool.tensor_tensor(
                        sq_sb[:, c0:c1, :], wt_sb[:, c0:c1, :],
                        wt_sb[:, c0:c1, :], mybir.AluOpType.mult,
                    )
                s2b = small.tile([P, 2], bf16)
                for j in range(2):
                    pool.tensor_tensor(
                        s2b[:, j : j + 1], styles[:], styles[:],
                        mybir.AluOpType.mult,
                    )

                vcol_ps = spsum.tile([P, 2], f32, tag="vc")
                for k in range(TAPS):
                    nc.tensor.matmul(
                        vcol_ps[:], sq_sb[:, k, :], s2b[:],
                        start=(k == 0), stop=(k == TAPS - 1),
                    )

                # S_col = sqrt(2) * rsqrt(v + 1e-8)  (per-partition ACT scale)
                veps = small.tile([P, 1], f32)
                nc.scalar.activation(
                    veps[:], vcol_ps[:, :1], AF.Identity, bias=epsc_sb
                )
                vrec = small.tile([P, 1], f32)
                nc.vector.reciprocal(vrec[:], veps[:])
                nc.scalar.activation(
                    s_col[:], vrec[:], AF.Sqrt, bias=zc_sb, scale=LRELU_GAIN**2
                )

            # ---- main conv loop: variable input chunks (double-buffered);
            # the first chunk is small so PE starts sooner ----
            def conv_tile_hi(xt_flat, n0, off, width):
                """Open a PSUM accumulation group for `width` outputs centred
                at padded offset n0: all products that need only w_hi."""
                nrows = width // RES
                inner = ([ROW, nrows], [1, RES])
                # noise first: no PSUM dependency, so the epilogue can fire
                # the moment the accumulation group closes
                nz_bc = nzp.tile([P, 1, width], f32, tag="nz")
                nc.sync.dma_start(
                    nz_bc[:], nz[:, off : off + width].partition_broadcast(P)
                )
                pool.tensor_scalar_mul(nz_bc[:], nz_bc[:], nsg[:])

                pt = cpsum.tile([P, width], f32, tag="conv")
                # 13 hi-hi tap pairs, tap26 hi*(lo+hi), 2 x_lo pairs for taps
                # 22-25 (all need only w_hi), then 22 hi/lo correction pairs
                # (need w_lo, off the startup critical path)
                for i in range(13):
                    q0 = n0 + TAP_OFF[2 * i]
                    dq = TAP_OFF[2 * i + 1] - TAP_OFF[2 * i]
                    nc.tensor.matmul(
                        pt[:], wq[:, 2 * i : 2 * i + 2, :],
                        _pair_ap(xt_flat, NSLABC + q0, dq, inner),
                        start=(i == 0), stop=False, perf_mode=DRMODE,
                    )
                nc.tensor.matmul(
                    pt[:], _pair_ap(wq_flat, 26 * P, 0, ([1, P],)),
                    _pair_ap(xt_flat, n0 + TAP_OFF[26], NSLABC, inner),
                    start=False, stop=False, perf_mode=DRMODE,
                )
                for k in (18, 20, 22, 24):
                    qk = n0 + TAP_OFF[k]
                    dq = TAP_OFF[k + 1] - TAP_OFF[k]
                    nc.tensor.matmul(
                        pt[:], wq[:, k : k + 2, :],
                        _pair_ap(xt_flat, qk, dq, inner),
                        start=False, stop=False, perf_mode=DRMODE,
                    )
                return pt, nz_bc

            def conv_tile_corr(pt, xt_flat, n0, width):
                """Close the group: w_lo correction pairs (need w_lo)."""
                nrows = width // RES
                inner = ([ROW, nrows], [1, RES])
                for k in range(NLO):
                    qk = n0 + TAP_OFF[k]
                    nc.tensor.matmul(
                        pt[:], _pair_ap(wq_flat, k * P, TAPS * P, ([1, P],)),
                        _pair_ap(xt_flat, qk, NSLABC, inner),
                        start=False, stop=(k == NLO - 1), perf_mode=DRMODE,
                    )

            def conv_tile_epi(pt, nz_bc, off, width):
                ut = outp.tile([P, width], f32, tag="out")
                # ut = psum * (dcoef*sqrt2) + noise_term
                nc.vector.scalar_tensor_tensor(
                    ut[:], pt[:], s_col[:], nz_bc[:, 0, :],
                    mybir.AluOpType.mult, mybir.AluOpType.add,
                )
                nc.scalar.activation(
                    ut[:], ut[:], AF.Prelu,
                    bias=b_col[:], scale=1.0, alpha=acol_sb,
                )
                nc.vector.tensor_scalar(
                    ut[:], ut[:], CLAMP, -CLAMP,
                    mybir.AluOpType.min, mybir.AluOpType.max,
                )
                nc.sync.dma_start(y[:, off : off + width], ut[:])

            def conv_tile(xt_flat, n0, off, width):
                pt, nz_bc = conv_tile_hi(xt_flat, n0, off, width)
                conv_tile_corr(pt, xt_flat, n0, width)
                conv_tile_epi(pt, nz_bc, off, width)

            for ci, (a, n) in enumerate(CHUNKS):
                last_chunk = ci == len(CHUNKS) - 1
                if ci == 0:
                    xt = xt1  # chunk 1 was DMA'd during startup
                else:
                    xt = xchunk.tile([P, 2, NSLABC], fp8, tag="xchunk")
                    s_g = (a - 1) * SLICE  # chunk start in the padded slab
                    wlen = LEAD + (n + 2) * SLICE + 46
                    for sl in (1, 0):
                        for i0 in range(0, wlen, step):
                            bnd = min(wlen, i0 + step)
                            nc.sync.dma_start(
                                xt[:, sl, i0:bnd],
                                (xhi if sl else xlo)[:, s_g + i0 : s_g + bnd],
                            )
                xt_flat = xt[:].rearrange("p a b -> p (a b)")
                if ci == 0:
                    # defer the first chunk's epilogues until after the demod
                    # block: puts the vcol matmul late enough in the PE stream
                    # that its inputs are ready, with no read-before-write on
                    # s_col
                    n0a = LEAD + SLICE
                    n0b = n0a + 16 * ROW
                    ptA, nzA = conv_tile_hi(xt_flat, n0a, 0, NCHUNK)
                    ptB, nzB = conv_tile_hi(xt_flat, n0b, NCHUNK, NCHUNK)
                    conv_tile_corr(ptA, xt_flat, n0a, NCHUNK)
                    conv_tile_corr(ptB, xt_flat, n0b, NCHUNK)
                    deferred = [((ptA, nzA), 0), ((ptB, nzB), NCHUNK)]
                    demod_block()
                    for (pt, nz_bc), off in deferred:
                        conv_tile_epi(pt, nz_bc, off, NCHUNK)
                    continue
                for dl in range(1, n + 1):       # local padded slice index
                    d = a + dl - 1               # global padded slice index
                    for half in range(2):        # 16 rows each
                        n0 = LEAD + dl * SLICE + half * 16 * ROW
                        off = (d - 1) * 1024 + half * NCHUNK
                        if last_chunk and dl == n and half == 1:
                            # split the final tile so the tail drain is short
                            conv_tile(xt_flat, n0, off, 384)
                            conv_tile(xt_flat, n0 + 12 * ROW, off + 384, 128)
                        else:
                            conv_tile(xt_flat, n0, off, NCHUNK)

    nc.compile()
    return nc


def _get_nc():
    global _NC_CACHE
    if _NC_CACHE is None:
        _NC_CACHE = build_nc()
    return _NC_CACHE


def _make_core_inputs(x, w, affine_weight, affine_bias, weight, noise_const,
                      noise_strength, bias):
    """Build the 8 per-core input maps (host-side sharding / layout only)."""
    aff_host = np.ascontiguousarray(
        affine_weight.T.reshape(4, P, P).transpose(1, 0, 2)
    )  # [wd_p, j, ci]
    wt_host = np.ascontiguousarray(
        weight.reshape(P, P, TAPS).transpose(1, 2, 0)
    ).astype(ml_dtypes.bfloat16)  # [ci, k, co]
    sm_host = np.zeros((P, 8), np.float32)
    sm_host[:, 0] = affine_bias
    sm_host[:, 1] = bias
    sm_host[:, 2] = float(noise_strength.reshape(-1)[0])
    sm_host[:, 3] = LRELU_ALPHA
    sm_host[:, 4] = 1e-8
    sm_host[:, 6:8] = 1.0

    in_maps = []
    for c in range(8):
        b, half = divmod(c, 2)
        d0 = DHALF * half
        slab = np.zeros((P, NSLAB), np.float32)
        view = slab[:, LEAD : LEAD + BODY].reshape(P, NSLICES, ROW, ROW)
        lo = max(0, d0 - 1)
        hi = min(RES, d0 + DHALF + 1)
        # padded slice s holds global slice d0-1+s
        view[:, lo - (d0 - 1) : hi - (d0 - 1), :RES, :RES] = x[b, :, lo:hi]
        slab_hi = slab.astype(E4)
        slab_lo = (slab - slab_hi.astype(np.float32)).astype(E4)
        nz_host = np.ascontiguousarray(
            noise_const[d0 : d0 + DHALF].reshape(1, NOUT)
        )
        affwv_host = np.zeros((P, 520), np.float32)
        affwv_host[:, :512] = aff_host.reshape(P, 512)
        affwv_host[:, 512:516] = w[b].reshape(4, P).T
        affwv_host[:, 517:519] = 1.0
        in_maps.append({
            "xhi": slab_hi,
            "xlo": slab_lo,
            "wt": wt_host,
            "affwv": affwv_host.astype(ml_dtypes.bfloat16),
            "sm": sm_host,
            "nz": nz_host,
        })
    return in_maps


def kernel(x, w, affine_weight, affine_bias, weight, noise_const,
           noise_strength, bias):
    global LAST_EXEC_NS
    x = np.asarray(x, np.float32)
    w = np.asarray(w, np.float32)
    affine_weight = np.asarray(affine_weight, np.float32)
    affine_bias = np.asarray(affine_bias, np.float32)
    weight = np.asarray(weight, np.float32)
    noise_const = np.asarray(noise_const, np.float32)
    noise_strength = np.asarray(noise_strength, np.float32)
    bias = np.asarray(bias, np.float32)

    nc = _get_nc()
    in_maps = _make_core_inputs(
        x, w, affine_weight, affine_bias, weight, noise_const,
        noise_strength, bias,
    )
    trace = bool(os.environ.get("KERNEL_TRACE"))
    if trace:
        from concourse.bass_utils import axon_active

        if axon_active():
            try:  # axon NTFF capture needs the profile hook; absent in some pods
                from antenv.axon_hooks import get_axon_ntff_profile_hook  # noqa: F401
            except ImportError:
                trace = False
    res = run_bass_kernel_spmd(nc, in_maps, core_ids=list(range(8)), trace=trace)
    LAST_EXEC_NS = res.exec_time_ns

    out = np.empty((B, P, RES, RES, RES), np.float32)
    for c in range(8):
        b, half = divmod(c, 2)
        d0 = DHALF * half
        out[b, :, d0 : d0 + DHALF] = res.results[c]["y"].reshape(
            P, DHALF, RES, RES
        )
    return out


# revision 22
# speedup vs baseline: 1.0064x; 1.0064x over previous
"""TRN2 Bass kernel for nn_SynthesisLayer (StyleGAN-style modulated 3D conv).

Math: for each sample b
  styles = w[b] @ affine_weight.T / sqrt(512) + affine_bias          [Cin]
  wmod   = weight * styles[None,:,None]                              [Co,Ci,27]
  dcoef  = rsqrt(sum_{ci,k} wmod^2 + 1e-8)                           [Co]
  y      = dcoef * conv3d(x[b], wmod, pad=1) + noise_const*ns + bias
  out    = clip(lrelu(y)*sqrt(2), -256, 256)

Device implementation (per core):
  - conv3d = shifted matmuls (contraction over Cin=128 on partitions)
    accumulated in PSUM, weights modulated on device by styles.
  - fp8 DoubleRow path: x is pre-split on the host into e4m3 hi+lo slabs;
    modulated (bf16-shipped) weights are split on device into e4m3 hi+lo.
    Each DoubleRow matmul computes two (weight, shifted-x) products per
    PSUM row at 0.5 cycles/row.  Per output element: 27 hi*hi products +
    27 w_hi*x_lo + 18 w_lo*x_hi corrections (w_lo of taps 18-26 dropped,
    rel err ~1.7% vs the 2e-2 budget) = 72 products = 36 matmuls per
    512-wide tile, vs 27 full-rate fp32r matmuls for the exact conv.
  - demod + noise fold into a per-partition scale (ACT Prelu) and a DVE
    scalar_tensor_tensor; demod squares/reductions run on the Pool engine
    off the DVE critical path.

Sharding: 8 cores = 4 samples x 2 D-halves. Each core gets zero-padded
fp8 hi/lo input slabs [128, NSLAB] (33-wide rows, 33-row slices, one-slice
D halo), computes output [128, 16*32*32], host reassembles. No collectives.
"""

import math
import os
import sys

for _p in ("/opt/trn_rl_repo", "/root/.axon_site/_ro/trn_rl_repo"):
    if os.path.isdir(_p) and _p not in sys.path:
        sys.path.insert(0, _p)

import numpy as np
import ml_dtypes

import concourse.mybir as mybir
from concourse import bacc
from concourse.ap import AP
from concourse.tile import TileContext
from concourse.bass_utils import run_bass_kernel_spmd

P = 128          # Cin = Cout = 128
TAPS = 27        # 3x3x3
NDROP = 9        # taps whose w_lo correction is dropped (18..26)
NLO = TAPS - NDROP
RES = 32
B = 4
W_DIM = 512
ROW = 33         # padded row width  (32 real + 1 zero)
SLICE = ROW * ROW  # 1089 padded slice (32 real rows + 1 zero row)
LEAD = 34        # leading zero guard (one row + one elem)
NSLICES = 18     # 16 output slices + 1 halo each side
BODY = NSLICES * SLICE
NSLAB = LEAD + BODY + 46   # tail guard; max AP end = 19637
DHALF = 16                 # output D slices per core
NOUT = DHALF * RES * RES   # 16384
DCH = 4                    # output D slices per pipelined input chunk
NSLABC = LEAD + (DCH + 2) * SLICE + 46  # 6614: chunk tile incl. halo+guards
NCHUNK = 512               # psum tile free size (one PSUM bank of fp32)
LRELU_ALPHA = 0.2
LRELU_GAIN = math.sqrt(2.0)
CLAMP = 256.0

f32 = mybir.dt.float32
f32r = mybir.dt.float32r
bf16 = mybir.dt.bfloat16
fp8 = mybir.dt.float8e4
DRMODE = mybir.MatmulPerfMode.DoubleRow
AF = mybir.ActivationFunctionType
E4 = ml_dtypes.float8_e4m3fn

# tap k = kd*9 + kh*3 + kw; shift of tap k relative to the tile center
TAP_OFF = [
    (kd - 1) * SLICE + (kh - 1) * ROW + (kw - 1)
    for kd in range(3) for kh in range(3) for kw in range(3)
]

_NC_CACHE = None
LAST_EXEC_NS = None


def _pair_ap(flat_ap, off, delta, inner_dims):
    """[[p],[delta,2],*inner_dims] AP at element offset `off` of a 2D AP."""
    dims = [list(flat_ap.ap[0]), [delta, 2]] + [list(d) for d in inner_dims]
    return AP(flat_ap.tensor, flat_ap.offset + off, dims)


def build_nc():
    nc = bacc.Bacc("TRN2", target_bir_lowering=False, debug=False, num_devices=8)
    pool = nc.engines[mybir.EngineType.Pool]

    xhi = nc.dram_tensor("xhi", [P, NSLAB], fp8, kind="ExternalInput")
    xlo = nc.dram_tensor("xlo", [P, NSLAB], fp8, kind="ExternalInput")
    wt = nc.dram_tensor("wt", [P, TAPS, P], bf16, kind="ExternalInput")
    # affwv = aff (4*128) | wv (5) | pad, one DMA for the whole style path
    affwv = nc.dram_tensor("affwv", [P, 520], bf16, kind="ExternalInput")
    # sm cols: 0=affine_bias 1=bias 2=noise_strength 3=lrelu_alpha 4=eps
    #          5=zero 6,7=ones
    sm = nc.dram_tensor("sm", [P, 8], f32, kind="ExternalInput")
    nz = nc.dram_tensor("nz", [1, NOUT], f32, kind="ExternalInput")
    y = nc.dram_tensor("y", [P, NOUT], f32, kind="ExternalOutput")

    with TileContext(nc) as tc:
        with (
            tc.tile_pool(name="big", bufs=1) as big,
            tc.tile_pool(name="small", bufs=1) as small,
            tc.tile_pool(name="nzp", bufs=6) as nzp,
            tc.tile_pool(name="xchunk", bufs=2) as xchunk,
            tc.tile_pool(name="outp", bufs=4) as outp,
            tc.tile_pool(name="cpsum", bufs=6, space="PSUM") as cpsum,
            tc.tile_pool(name="spsum", bufs=1, space="PSUM") as spsum,
        ):
            # dummy activation with no DMA deps: hoists the one-time
            # LoadActFuncSet (~1.3us) off the styles critical path
            dummy = small.tile([P, 1], f32)
            pool.memset(dummy[:], 0.0)
            nc.scalar.activation(dummy[:], dummy[:], AF.Identity, bias=dummy[:])
            nc.scalar.activation(dummy[:], dummy[:], AF.Sqrt, bias=dummy[:])
            nc.scalar.activation(
                dummy[:], dummy[:], AF.Prelu, bias=dummy[:], scale=1.0,
                alpha=dummy[:],
            )

            # ---- merged small loads first: the style path comes off these ----
            affwv_sb = small.tile([P, 520], bf16)
            nc.sync.dma_start(affwv_sb[:], affwv[:])
            sm_sb = small.tile([P, 8], f32)
            nc.sync.dma_start(sm_sb[:], sm[:])
            ab_sb = sm_sb[:, 0:1]
            bb_sb = sm_sb[:, 1:2]
            nsb_sb = sm_sb[:, 2:3]
            acol_sb = sm_sb[:, 3:4]
            epsc_sb = sm_sb[:, 4:5]
            zc_sb = sm_sb[:, 5:6]

            # weight DMA in tap chunks so quantization starts early
            wt_sb = big.tile([P, TAPS, P], bf16)
            WCH = [(0, 2), (2, 9), (9, 16), (16, 23), (23, TAPS)]
            CHUNKS = [(1, 1), (2, 3), (5, 4), (9, 4), (13, 4)]  # (a, n_out)
            xt1 = xchunk.tile([P, 2, NSLABC], fp8, tag="xchunk")
            wlen1 = LEAD + 3 * SLICE + 46
            step = 2 * SLICE
            nc.sync.dma_start(wt_sb[:, 0:2, :], wt[:, 0:2, :])
            nc.sync.dma_start(xt1[:, 1, 0:step], xhi[:, 0:step])
            nc.sync.dma_start(xt1[:, 1, step:wlen1], xhi[:, step:wlen1])
            for c0, c1 in WCH[1:]:
                nc.sync.dma_start(wt_sb[:, c0:c1, :], wt[:, c0:c1, :])
            nc.sync.dma_start(xt1[:, 0, 0:step], xlo[:, 0:step])
            nc.sync.dma_start(xt1[:, 0, step:wlen1], xlo[:, step:wlen1])

            # ---- styles = w[b] @ aff.T / sqrt(512) + affine_bias ----
            # N=1 matmuls are ISA-illegal; use N=2 and read col 0
            st_ps = spsum.tile([P, 2], f32, tag="st")
            for j in range(4):
                nc.tensor.matmul(
                    st_ps[:], affwv_sb[:, j * P : (j + 1) * P],
                    affwv_sb[:, 512 + j : 514 + j],
                    start=(j == 0), stop=(j == 3),
                )
            styles = small.tile([P, 1], f32)
            nc.scalar.activation(
                styles[:], st_ps[:, :1], AF.Identity,
                bias=ab_sb, scale=1.0 / math.sqrt(W_DIM),
            )

            # ---- modulated weights, split into fp8 hi + lo ----
            # wq slots 0..26 = hi taps, 27..48 = lo taps 0..21
            wq = big.tile([P, TAPS + NLO, P], fp8)
            for c0, c1 in WCH:
                nc.vector.tensor_scalar_mul(
                    wq[:, c0:c1, :], wt_sb[:, c0:c1, :], styles[:]
                )
            for c0, c1 in ((0, 6), (6, 12), (12, NLO)):
                nc.vector.scalar_tensor_tensor(
                    wq[:, TAPS + c0 : TAPS + c1, :], wt_sb[:, c0:c1, :],
                    styles[:], wq[:, c0:c1, :],
                    mybir.AluOpType.mult, mybir.AluOpType.subtract,
                )
            wq_flat = wq[:].rearrange("p a b -> p (a b)")

            # B_col = bias * sqrt(2)
            b_col = small.tile([P, 1], f32)
            nc.vector.tensor_scalar_mul(b_col[:], bb_sb, LRELU_GAIN)
            # noise gain = noise_strength * sqrt(2), per partition
            nsg = small.tile([P, 1], f32)
            nc.vector.tensor_scalar_mul(nsg[:], nsb_sb, LRELU_GAIN)
            s_col = small.tile([P, 1], f32)

            def demod_block():
                # ---- demod sums: v[co] = sum_ci styles^2 * (sum_k wt^2),
                # via 27 tiny PE matmuls sq[:,k,:]^T @ s2 accumulating in PSUM
                # (no DVE reduce on the critical path).  Issued after chunk 1
                # so the PE's in-order stream reaches these matmuls only once
                # their inputs are long since ready; the first epilogue waits
                # on s_col, covered by PSUM buffering
                sq_sb = big.tile([P, TAPS, P], bf16)  # [ci, k, co]
                for c0, c1 in WCH:
                    pool.tensor_tensor(
                        sq_sb[:, c0:c1, :], wt_sb[:, c0:c1, :],
                        wt_sb[:, c0:c1, :], mybir.AluOpType.mult,
                    )
                s2b = small.tile([P, 2], bf16)
                for j in range(2):
                    pool.tensor_tensor(
                        s2b[:, j : j + 1], styles[:], styles[:],
                        mybir.AluOpType.mult,
                    )

                vcol_ps = spsum.tile([P, 2], f32, tag="vc")
                for k in range(TAPS):
                    nc.tensor.matmul(
                        vcol_ps[:], sq_sb[:, k, :], s2b[:],
                        start=(k == 0), stop=(k == TAPS - 1),
                    )

                # S_col = sqrt(2) * rsqrt(v + 1e-8)  (per-partition ACT scale)
                veps = small.tile([P, 1], f32)
                nc.scalar.activation(
                    veps[:], vcol_ps[:, :1], AF.Identity, bias=epsc_sb
                )
                vrec = small.tile([P, 1], f32)
                nc.vector.reciprocal(vrec[:], veps[:])
                nc.scalar.activation(
                    s_col[:], vrec[:], AF.Sqrt, bias=zc_sb, scale=LRELU_GAIN**2
                )

            # ---- main conv loop: variable input chunks (double-buffered);
            # the first chunk is small so PE starts sooner ----
            def conv_tile_hi(xt_flat, n0, off, width):
                """Open a PSUM accumulation group for `width` outputs centred
                at padded offset n0: all products that need only w_hi."""
                nrows = width // RES
                inner = ([ROW, nrows], [1, RES])
                # noise first: no PSUM dependency, so the epilogue can fire
                # the moment the accumulation group closes
                nz_bc = nzp.tile([P, 1, width], f32, tag="nz")
                nc.sync.dma_start(
                    nz_bc[:], nz[:, off : off + width].partition_broadcast(P)
                )
                pool.tensor_scalar_mul(nz_bc[:], nz_bc[:], nsg[:])

                pt = cpsum.tile([P, width], f32, tag="conv")
                # 13 hi-hi tap pairs, tap26 hi*(lo+hi), 2 x_lo pairs for taps
                # 22-25 (all need only w_hi), then 22 hi/lo correction pairs
                # (need w_lo, off the startup critical path)
                for i in range(13):
                    q0 = n0 + TAP_OFF[2 * i]
                    dq = TAP_OFF[2 * i + 1] - TAP_OFF[2 * i]
                    nc.tensor.matmul(
                        pt[:], wq[:, 2 * i : 2 * i + 2, :],
                        _pair_ap(xt_flat, NSLABC + q0, dq, inner),
                        start=(i == 0), stop=False, perf_mode=DRMODE,
                    )
                nc.tensor.matmul(
                    pt[:], _pair_ap(wq_flat, 26 * P, 0, ([1, P],)),
                    _pair_ap(xt_flat, n0 + TAP_OFF[26], NSLABC, inner),
                    start=False, stop=False, perf_mode=DRMODE,
                )
                for k in (18, 20, 22, 24):
                    qk = n0 + TAP_OFF[k]
                    dq = TAP_OFF[k + 1] - TAP_OFF[k]
                    nc.tensor.matmul(
                        pt[:], wq[:, k : k + 2, :],
                        _pair_ap(xt_flat, qk, dq, inner),
                        start=False, stop=False, perf_mode=DRMODE,
                    )
                return pt, nz_bc

            def conv_tile_corr(pt, xt_flat, n0, width):
                """Close the group: w_lo correction pairs (need w_lo)."""
                nrows = width // RES
                inner = ([ROW, nrows], [1, RES])
                for k in range(NLO):
                    qk = n0 + TAP_OFF[k]
                    nc.tensor.matmul(
                        pt[:], _pair_ap(wq_flat, k * P, TAPS * P, ([1, P],)),
                        _pair_ap(xt_flat, qk, NSLABC, inner),
                        start=False, stop=(k == NLO - 1), perf_mode=DRMODE,
                    )

            def conv_tile_epi(pt, nz_bc, off, width):
                ut = outp.tile([P, width], f32, tag="out")
                # ut = psum * (dcoef*sqrt2) + noise_term
                nc.vector.scalar_tensor_tensor(
                    ut[:], pt[:], s_col[:], nz_bc[:, 0, :],
                    mybir.AluOpType.mult, mybir.AluOpType.add,
                )
                nc.scalar.activation(
                    ut[:], ut[:], AF.Prelu,
                    bias=b_col[:], scale=1.0, alpha=acol_sb,
                )
                nc.vector.tensor_scalar(
                    ut[:], ut[:], CLAMP, -CLAMP,
                    mybir.AluOpType.min, mybir.AluOpType.max,
                )
                nc.sync.dma_start(y[:, off : off + width], ut[:])

            def conv_tile(xt_flat, n0, off, width):
                pt, nz_bc = conv_tile_hi(xt_flat, n0, off, width)
                conv_tile_corr(pt, xt_flat, n0, width)
                conv_tile_epi(pt, nz_bc, off, width)

            for ci, (a, n) in enumerate(CHUNKS):
                last_chunk = ci == len(CHUNKS) - 1
                if ci == 0:
                    xt = xt1  # chunk 1 was DMA'd during startup
                else:
                    xt = xchunk.tile([P, 2, NSLABC], fp8, tag="xchunk")
                    s_g = (a - 1) * SLICE  # chunk start in the padded slab
                    wlen = LEAD + (n + 2) * SLICE + 46
                    for sl in (1, 0):
                        for i0 in range(0, wlen, step):
                            bnd = min(wlen, i0 + step)
                            nc.sync.dma_start(
                                xt[:, sl, i0:bnd],
                                (xhi if sl else xlo)[:, s_g + i0 : s_g + bnd],
                            )
                xt_flat = xt[:].rearrange("p a b -> p (a b)")
                if ci == 0:
                    # defer the first chunk's epilogues until after the demod
                    # block: puts the vcol matmul late enough in the PE stream
                    # that its inputs are ready, with no read-before-write on
                    # s_col
                    n0a = LEAD + SLICE
                    n0b = n0a + 16 * ROW
                    ptA, nzA = conv_tile_hi(xt_flat, n0a, 0, NCHUNK)
                    ptB, nzB = conv_tile_hi(xt_flat, n0b, NCHUNK, NCHUNK)
                    conv_tile_corr(ptA, xt_flat, n0a, NCHUNK)
                    conv_tile_corr(ptB, xt_flat, n0b, NCHUNK)
                    deferred = [((ptA, nzA), 0), ((ptB, nzB), NCHUNK)]
                    demod_block()
                    for (pt, nz_bc), off in deferred:
                        conv_tile_epi(pt, nz_bc, off, NCHUNK)
                    continue
                for dl in range(1, n + 1):       # local padded slice index
                    d = a + dl - 1               # global padded slice index
                    for half in range(2):        # 16 rows each
                        n0 = LEAD + dl * SLICE + half * 16 * ROW
                        off = (d - 1) * 1024 + half * NCHUNK
                        if last_chunk and dl == n and half == 1:
                            # split the final tile so the tail drain is short
                            conv_tile(xt_flat, n0, off, 384)
                            conv_tile(xt_flat, n0 + 12 * ROW, off + 384, 128)
                        else:
                            conv_tile(xt_flat, n0, off, NCHUNK)

    nc.compile()
    return nc


def _get_nc():
    global _NC_CACHE
    if _NC_CACHE is None:
        _NC_CACHE = build_nc()
    return _NC_CACHE


def _make_core_inputs(x, w, affine_weight, affine_bias, weight, noise_const,
                      noise_strength, bias):
    """Build the 8 per-core input maps (host-side sharding / layout only)."""
    aff_host = np.ascontiguousarray(
        affine_weight.T.reshape(4, P, P).transpose(1, 0, 2)
    )  # [wd_p, j, ci]
    wt_host = np.ascontiguousarray(
        weight.reshape(P, P, TAPS).transpose(1, 2, 0)
    ).astype(ml_dtypes.bfloat16)  # [ci, k, co]
    sm_host = np.zeros((P, 8), np.float32)
    sm_host[:, 0] = affine_bias
    sm_host[:, 1] = bias
    sm_host[:, 2] = float(noise_strength.reshape(-1)[0])
    sm_host[:, 3] = LRELU_ALPHA
    sm_host[:, 4] = 1e-8
    sm_host[:, 6:8] = 1.0

    in_maps = []
    for c in range(8):
        b, half = divmod(c, 2)
        d0 = DHALF * half
        slab = np.zeros((P, NSLAB), np.float32)
        view = slab[:, LEAD : LEAD + BODY].reshape(P, NSLICES, ROW, ROW)
        lo = max(0, d0 - 1)
        hi = min(RES, d0 + DHALF + 1)
        # padded slice s holds global slice d0-1+s
        view[:, lo - (d0 - 1) : hi - (d0 - 1), :RES, :RES] = x[b, :, lo:hi]
        slab_hi = slab.astype(E4)
        slab_lo = (slab - slab_hi.astype(np.float32)).astype(E4)
        nz_host = np.ascontiguousarray(
            noise_const[d0 : d0 + DHALF].reshape(1, NOUT)
        )
        affwv_host = np.zeros((P, 520), np.float32)
        affwv_host[:, :512] = aff_host.reshape(P, 512)
        affwv_host[:, 512:516] = w[b].reshape(4, P).T
        affwv_host[:, 517:519] = 1.0
        in_maps.append({
            "xhi": slab_hi,
            "xlo": slab_lo,
            "wt": wt_host,
            "affwv": affwv_host.astype(ml_dtypes.bfloat16),
            "sm": sm_host,
            "nz": nz_host,
        })
    return in_maps


def kernel(x, w, affine_weight, affine_bias, weight, noise_const,
           noise_strength, bias):
    global LAST_EXEC_NS
    x = np.asarray(x, np.float32)
    w = np.asarray(w, np.float32)
    affine_weight = np.asarray(affine_weight, np.float32)
    affine_bias = np.asarray(affine_bias, np.float32)
    weight = np.asarray(weight, np.float32)
    noise_const = np.asarray(noise_const, np.float32)
    noise_strength = np.asarray(noise_strength, np.float32)
    bias = np.asarray(bias, np.float32)

    nc = _get_nc()
    in_maps = _make_core_inputs(
        x, w, affine_weight, affine_bias, weight, noise_const,
        noise_strength, bias,
    )
    trace = bool(os.environ.get("KERNEL_TRACE"))
    if trace:
        from concourse.bass_utils import axon_active

        if axon_active():
            try:  # axon NTFF capture needs the profile hook; absent in some pods
                from antenv.axon_hooks import get_axon_ntff_profile_hook  # noqa: F401
            except ImportError:
                trace = False
    res = run_bass_kernel_spmd(nc, in_maps, core_ids=list(range(8)), trace=trace)
    LAST_EXEC_NS = res.exec_time_ns

    out = np.empty((B, P, RES, RES, RES), np.float32)
    for c in range(8):
        b, half = divmod(c, 2)
        d0 = DHALF * half
        out[b, :, d0 : d0 + DHALF] = res.results[c]["y"].reshape(
            P, DHALF, RES, RES
        )
    return out
